# revision 24
# baseline (speedup 1.0000x reference)
"""Cosine-similarity attention kernel for Trainium2 (8 NeuronCores, SPMD).

Problem: B=4, D=1024, T=2048, n_head=8, alpha=5.0.
Math (per batch b, head h, with d = D/8 = 128):
    qn = l2norm(q, axis=d); kn = l2norm(k, axis=d)
    S  = alpha * qn^T kn          [Tq, Tk]
    P  = softmax(S, axis=Tk)
    out= v @ P^T                  [dv, Tq]

Sharding: head-parallel — the 32 (b, h) pairs are split 4-per-core across
8 cores. Each core computes full attention for its 4 pairs.

v2 design (PE-roofline oriented; all matmul streams fp16):
  - PE runs ONLY the scores and AV matmuls (the 2*T^2*d irreducible work)
    plus one 512-col rowsum finisher per q-block. The baseline's rowsum
    matmul (1/3 of PE time) is replaced by a DVE fp16 tree-reduction over
    exp tiles; the baseline's sum-of-squares matmuls are replaced by DVE
    square+reduce on host-transposed copies of q/k.
  - Scores computed TRANSPOSED (S^T = kn^T qn, [k, q] layout) so AV
    contracts over k on the partition dim. Softmax max-subtraction skipped:
    |S| <= alpha = 5, exp in [e-5, e5], safe in fp32 psum / fp16 out.
  - l2norm: ssq = DVE reduce over xT tiles -> [128,16]; rsqrt as
    exp(-0.5*ln(x)) on ACT (tiny tiles; Ln+Exp share the pinned table);
    inverse norms DMA-transposed to a [1, T] row and broadcast-multiplied
    (alpha folded into the q side).
  - Host pre-converts inputs to fp16 and pre-transposes q/k/v, halving
    input DMA traffic.
"""

import math
import os
import sys
from contextlib import ExitStack

for _p in ("/opt/trn_rl_repo", "/root/.axon_site/_ro/trn_rl_repo"):
    if os.path.isdir(_p) and _p not in sys.path:
        sys.path.insert(0, _p)

import numpy as np

import concourse.bass as bass
import concourse.tile as tile
from concourse import bacc, mybir
from concourse.bass_utils import run_bass_kernel_spmd

N_CORES = 8
B, DFULL, T = 4, 1024, 2048
NHEAD = 8
D = DFULL // NHEAD          # 128 per-head channels
PAIRS = (B * NHEAD) // N_CORES  # 4 (b, h) pairs per core
ALPHA = 5.0

NKT = T // 128              # 16 k-tiles of 128
QB = 512                    # q-block width
NQB = T // QB               # 4 q-blocks
CK = 2                      # k-tiles per exp chunk ([128, CK*512] psum chunk)
NC_ = NKT // CK             # 8 chunks per q-block

F32 = mybir.dt.float32
F16 = mybir.dt.float16
EXP = mybir.ActivationFunctionType.Exp
LN = mybir.ActivationFunctionType.Ln


class _PinnedActBacc(bacc.Bacc):
    """Bacc whose activation-table chooser is pinned so Exp and Ln both
    resolve to natural_log_exp_and_others (the default chooser would insert
    a ~1.3us table load at every Ln/Exp alternation)."""

    def insert_act_table_loads(self):
        import bass_rust as _bass_rust
        from concourse.hw_specs import get_activation_tables

        has_activation = any(
            isinstance(i, mybir.InstActivation)
            for b in self.main_func.blocks
            for i in b.instructions
        )
        if not has_activation:
            return
        keep = "natural_log_exp_and_others"
        drop = {
            mybir.ActivationFunctionType.Exp,
            mybir.ActivationFunctionType.Ln,
        }
        tables = []
        for name, fns in get_activation_tables(self.m.arch).items():
            tables.append((name, fns if name == keep else (fns - drop)))
        _bass_rust.insert_act_table_loads(self, tables)


def _build_nc(repeat: int = 1) -> bass.Bass:
    nc = _PinnedActBacc(None, target_bir_lowering=False)
    q_d = nc.declare_dram_parameter("q", [PAIRS, D, T], F16, isOutput=False)
    k_d = nc.declare_dram_parameter("k", [PAIRS, D, T], F16, isOutput=False)
    qt_d = nc.declare_dram_parameter("qt", [PAIRS, T, D], F16, isOutput=False)
    kt_d = nc.declare_dram_parameter("kt", [PAIRS, T, D], F16, isOutput=False)
    vt_d = nc.declare_dram_parameter("vt", [PAIRS, T, D + 1], F16, isOutput=False)
    out_d = nc.declare_dram_parameter("out", [PAIRS, T, D], F16, isOutput=True)

    with ExitStack() as ctx:
        ctx.enter_context(nc.allow_low_precision("fp16 attention streams"))
        tc = ctx.enter_context(tile.TileContext(nc))
        const_p = ctx.enter_context(tc.tile_pool(name="const", bufs=1))
        io_p = ctx.enter_context(tc.tile_pool(name="io", bufs=2))
        work_p = ctx.enter_context(tc.tile_pool(name="work", bufs=2))
        e_p = ctx.enter_context(tc.tile_pool(name="e", bufs=12))
        out_p = ctx.enter_context(tc.tile_pool(name="outp", bufs=3))
        # PSUM: chunks 2x[128,1024] f32 (4 banks) + 4x avt [128,129] (4)
        cps = ctx.enter_context(tc.tile_pool(name="cps", bufs=2, space="PSUM"))
        avps = ctx.enter_context(tc.tile_pool(name="avps", bufs=1, space="PSUM"))

        def emit_load(p):
            q_sb = io_p.tile([D, T], F16, tag="q")
            k_sb = io_p.tile([D, T], F16, tag="k")
            qt_sb = io_p.tile([128, NKT, D], F16, tag="qt")
            kt_sb = io_p.tile([128, NKT, D], F16, tag="kt")
            vt_sb = io_p.tile([128, NKT, D + 1], F16, tag="vt")
            # norm inputs first (they head the critical chain), vt last;
            # q-side on the SP queue, k-side on the DVE queue in parallel.
            # Device t-axis order is t' = (t%128)*16 + t//128 (host
            # permutes); the [T', d] tensors load with partition = tp =
            # t'//16 — one contiguous 4KB run per partition — so ssq lands
            # as [tp, tt] and the flat-row DMA below is natural-order.
            nc.sync.dma_start(
                out=qt_sb,
                in_=qt_d[p].rearrange("(tp t) d -> tp t d", tp=128),
            )
            nc.gpsimd.dma_start(
                out=kt_sb,
                in_=kt_d[p].rearrange("(tp t) d -> tp t d", tp=128),
            )
            nc.sync.dma_start(out=q_sb, in_=q_d[p])
            nc.gpsimd.dma_start(out=k_sb, in_=k_d[p])
            # v loads with partition = t' % 128 to match the scores psum
            # partition (k within a 128-tile).
            nc.sync.dma_start(
                out=vt_sb,
                in_=vt_d[p].rearrange("(t kp) d -> kp t d", kp=128),
            )
            return q_sb, k_sb, qt_sb, kt_sb, vt_sb

        I32 = mybir.dt.int32

        def emit_norms(q_sb, k_sb, qt_sb, kt_sb):
            qn = work_p.tile([D, T], F16, tag="qn")
            kn = work_p.tile([D, T], F16, tag="kn")
            for x16, xt, dst, tg in (
                (q_sb, qt_sb, qn, "q"),
                (k_sb, kt_sb, kn, "k"),
            ):
                sq = work_p.tile([128, NKT, D], F16, tag="sq")
                nc.vector.tensor_mul(sq, xt, xt)
                ssq = work_p.tile([128, NKT], F32, tag="ssq")
                nc.vector.tensor_reduce(
                    ssq, sq, axis=mybir.AxisListType.X, op=mybir.AluOpType.add
                )
                if tg == "q":
                    # fold alpha into q: rsqrt(ssq/alpha^2) = alpha/||q||
                    ssqs = work_p.tile([128, NKT], F32, tag="ssqs")
                    nc.vector.tensor_scalar(
                        ssqs, ssq, 1.0 / (ALPHA * ALPHA), None,
                        mybir.AluOpType.mult,
                    )
                    ssq = ssqs
                # rsqrt on DVE: quake seed + 2 Newton iterations (all
                # [128,16] — keeps the scalar engine free for exp)
                half = work_p.tile([128, NKT], I32, tag="half")
                nc.vector.tensor_scalar(
                    half, ssq.bitcast(I32), 1, None,
                    mybir.AluOpType.logical_shift_right,
                )
                y0i = work_p.tile([128, NKT], I32, tag="y0i")
                nc.vector.tensor_scalar(
                    y0i, half, 0x5F3759DF, -1,
                    mybir.AluOpType.subtract, mybir.AluOpType.mult,
                )
                y = y0i.bitcast(F32)
                for it in range(2):
                    h = work_p.tile([128, NKT], F32, tag=f"h{it}", name=f"h{it}")
                    nc.vector.tensor_mul(h, ssq, y)
                    h2 = work_p.tile([128, NKT], F32, tag=f"h2{it}", name=f"h2{it}")
                    nc.vector.tensor_mul(h2, h, y)
                    hs = work_p.tile([128, NKT], F32, tag=f"hs{it}", name=f"hs{it}")
                    nc.vector.tensor_scalar(
                        hs, h2, -0.5, 1.5,
                        mybir.AluOpType.mult, mybir.AluOpType.add,
                    )
                    yn = work_p.tile(
                        [128, NKT], F32 if it == 0 else F16,
                        tag=f"yn{it}", name=f"yn{it}",
                    )
                    nc.vector.tensor_mul(yn, y, hs)
                    y = yn
                inv16 = y
                # [tp, tt] -> flat [1, T'] row; t' = tp*16 + tt is exactly
                # the natural iteration order: a clean 128-descriptor DMA.
                invf = work_p.tile([1, T], F16, tag="invf_" + tg)
                eng = nc.gpsimd if tg == "q" else nc.sync
                eng.dma_start(out=invf, in_=inv16)
                # replicate the row to all 128 partitions (stride-0 free-dim
                # read) so the DVE mul below runs on plain per-lane data
                invrep = work_p.tile([128, T], F16, tag="invrep_" + tg)
                eng.dma_start(
                    out=invrep,
                    in_=invf.rearrange("p (o t) -> p o t", o=1).to_broadcast(
                        [1, 128, T]
                    ),
                )
                nc.vector.tensor_mul(dst, x16, invrep)
            return qn, kn

        total = PAIRS * repeat
        cur_load = emit_load(0)
        cur_norm = emit_norms(cur_load[0], cur_load[1], cur_load[2], cur_load[3])
        nxt_load = nxt_norm = None
        for p_rep in range(total):
            p = p_rep % PAIRS
            qn, kn = cur_norm
            vt_sb = cur_load[4]
            for qb in range(NQB):
                qsl = slice(qb * QB, (qb + 1) * QB)
                # transposed-output AV: avt[qt] [128q, 129] accumulates
                # e^T @ [v | 1] over k — col 128 is the softmax denominator
                avt = [
                    avps.tile(
                        [128, D + 1], F32, tag=f"avt{i}", name=f"avt{i}"
                    )
                    for i in range(4)
                ]
                for c in range(NC_):
                    sp = cps.tile([128, CK * 512], F32, tag="chunk")
                    for j in range(CK):
                        kt = CK * c + j
                        nc.tensor.matmul(
                            sp[:, j * 512:(j + 1) * 512],
                            lhsT=kn[:, kt * 128:(kt + 1) * 128],
                            rhs=qn[:, qsl],
                            start=True, stop=True,
                        )
                    e_c = e_p.tile([128, CK * 512], F16, tag="e")
                    nc.scalar.activation(e_c, sp, EXP)
                    for j in range(CK):
                        kt = CK * c + j
                        for i in range(4):
                            nc.tensor.matmul(
                                avt[i],
                                lhsT=e_c[:, j * 512 + i * 128:j * 512 + (i + 1) * 128],
                                rhs=vt_sb[:, kt, :],
                                start=(kt == 0), stop=(kt == NKT - 1),
                            )
                o_qb = out_p.tile([128, 4, D], F16, tag="o")
                for i in range(4):
                    invr = out_p.tile([128, 1], F32, tag="invr")
                    nc.vector.reciprocal_approx_fast(
                        out=invr, in_=avt[i][:, D:D + 1]
                    )
                    nc.vector.tensor_scalar(
                        o_qb[:, i, :], avt[i][:, :D], invr, None,
                        mybir.AluOpType.mult,
                    )
                nc.gpsimd.dma_start(
                    out=out_d[p][qb * QB:(qb + 1) * QB, :].rearrange(
                        "(qt qp) d -> qp qt d", qp=128
                    ),
                    in_=o_qb,
                )
                if p_rep + 1 < total:
                    if qb == 0:
                        nxt_load = emit_load((p_rep + 1) % PAIRS)
                    elif qb == 1:
                        nxt_norm = emit_norms(
                            nxt_load[0], nxt_load[1], nxt_load[2], nxt_load[3]
                        )
            if p_rep + 1 < total:
                cur_load, cur_norm = nxt_load, nxt_norm

    nc.finalize()
    return nc


_NC_CACHE = None


def _get_nc() -> bass.Bass:
    global _NC_CACHE
    if _NC_CACHE is None:
        _NC_CACHE = _build_nc()
    return _NC_CACHE


def _permute_cols(x):
    """[P, D, T] logical -> device order t' = (t%128)*16 + t//128."""
    P, Dd, Tt = x.shape
    return x.reshape(P, Dd, Tt // 128, 128).transpose(0, 1, 3, 2).reshape(
        P, Dd, Tt
    )


def make_in_maps(q: np.ndarray, k: np.ndarray, v: np.ndarray):
    """Shard full [B, D, T] f32 inputs into per-core fp16 in_maps, with the
    t axis in device order t' = (t%128)*16 + t//128."""
    qr = _permute_cols(q.reshape(B * NHEAD, D, T)).astype(np.float16)
    kr = _permute_cols(k.reshape(B * NHEAD, D, T)).astype(np.float16)
    vr = _permute_cols(v.reshape(B * NHEAD, D, T)).astype(np.float16)
    qt = qr.transpose(0, 2, 1)
    kt = kr.transpose(0, 2, 1)
    # v transposed with a ones column appended: the AV matmul's col 128
    # accumulates the softmax denominator
    vt = np.concatenate(
        [
            vr.transpose(0, 2, 1),
            np.ones((B * NHEAD, T, 1), np.float16),
        ],
        axis=2,
    )
    in_maps = []
    for c in range(N_CORES):
        sl = slice(c * PAIRS, (c + 1) * PAIRS)
        in_maps.append({
            "q": np.ascontiguousarray(qr[sl]),
            "k": np.ascontiguousarray(kr[sl]),
            "qt": np.ascontiguousarray(qt[sl]),
            "kt": np.ascontiguousarray(kt[sl]),
            "vt": np.ascontiguousarray(vt[sl]),
        })
    return in_maps


def gather_out(results) -> np.ndarray:
    outs = np.concatenate(
        [results[c]["out"] for c in range(N_CORES)], axis=0
    )  # [32, T', d] f16, t' in device order
    # [32, tp(128), tt(16), d] -> [32, d, tt, tp] -> [32, d, T]
    outs = outs.reshape(B * NHEAD, 128, T // 128, D).transpose(0, 3, 2, 1)
    return np.ascontiguousarray(
        outs.reshape(B, DFULL, T), dtype=np.float32
    )


def run(q, k, v, **kwargs):
    nc = _get_nc()
    res = run_bass_kernel_spmd(nc, make_in_maps(q, k, v), list(range(N_CORES)), **kwargs)
    return gather_out(res.results), res


def kernel(q: np.ndarray, k: np.ndarray, v: np.ndarray) -> np.ndarray:
    out, _ = run(q, k, v)
    return out
